# revision 1
# baseline (speedup 1.0000x reference)
"""Chamfer-style point loss (nn_PointLoss) on 8 Trainium2 NeuronCores.

Math (reference): reflect points across plane n.x+d=0; half1 = reflected
points (valid where s=p.n+d < 0, mask m1), half2 = original points (mask
m2 = ~m1). D[i,j] = ||half1[i]-half2[j]||^2. Output scalar =
50*(sum_j min_i(D) m2_j / c2 + sum_i min_j(D) m1_i / c1).

v3 device formulation: F[i,j] = r1'[i] + r2'[j] - 2*a_i.b_j with penalty
P=2^14 on masked-out rows/cols. Row and column operand prep is merged
into one (128,36)-wide pass (cols appended as 4 extra columns) using
region-constant tiles; hi/lo bf16 splits write a single composite that is
scattered into K-major DRAM images with rearranged-destination DMAs
spread over the SP/Activation/Pool queues. One K=16 bf16 matmul per
(128,512) tile; tiles negated into fp16 so mins become maxes. Row-min
partials combined via one AllReduce(max) on a (128,33) f32 payload.

Sharding: half2 (column) axis split 8 ways, 512 cols/core; every core
holds all rows.
"""

import os
import sys

import numpy as np

for _p in ("/opt/trn_rl_repo", "/root/.axon_site/_ro/trn_rl_repo"):
    if os.path.isdir(_p) and _p not in sys.path:
        sys.path.insert(0, _p)

import concourse.bacc as bacc
import concourse.bass_isa as bass_isa
import concourse.tile as tile
from concourse import mybir
from concourse.bass_utils import run_bass_kernel_spmd

FP = mybir.dt.float32
BF = mybir.dt.bfloat16
HF = mybir.dt.float16
AX = mybir.AxisListType
OP = mybir.AluOpType

N = 4096
NCORES = 8
QT = 32            # row q-slots (q-inner layout: [p,q] is point 32p+q)
QC = QT // NCORES  # 4 col slots per partition (512 columns/core)
W = QT + QC        # merged row+col working width
PEN = float(2**14)  # keeps penalized F finite in fp16
BIG = 1.0e30
CMINIT = -60000.0


def _emit(tc, out_ap, norm_ap, pa_ap, oh_ap):
    nc = tc.nc

    psf = tc.alloc_tile_pool(name="psf", bufs=2, space="PSUM")
    pss = tc.alloc_tile_pool(name="pss", bufs=2, space="PSUM")
    per = tc.alloc_tile_pool(name="per", bufs=1)
    fsp = tc.alloc_tile_pool(name="fsp", bufs=3)
    drm = tc.alloc_tile_pool(name="drm", bufs=1, space="DRAM")

    def _t(shape, name, dt=FP):
        return per.tile(shape, dt, name=name)

    # ---- inputs to SBUF (split across both hwdge queues)
    norm_sb = _t([1, 4], "norm_sb")
    nc.sync.dma_start(norm_sb[:], norm_ap[:])
    PA = _t([128, W, 3], "PA")
    nc.sync.dma_start(PA[:], pa_ap[:])
    OH = _t([128, 1], "OH")
    nc.scalar.dma_start(OH[:], oh_ap[:])

    ones_c = _t([128, 1], "ones_c")
    nc.gpsimd.memset(ones_c[:], 1.0)
    ONES64 = _t([128, 64], "ONES64", BF)
    nc.gpsimd.memset(ONES64[:], 1.0)

    # region-constant tiles: first QT cols = row region, last QC = col region
    BETA = _t([128, W], "BETA")
    nc.gpsimd.memset(BETA[:, 0:QT], 1.0)
    nc.gpsimd.memset(BETA[:, QT:W], -2.0)
    SGNP = _t([128, W], "SGNP")
    nc.gpsimd.memset(SGNP[:, 0:QT], -PEN)
    nc.gpsimd.memset(SGNP[:, QT:W], PEN)
    ROWP = _t([128, W], "ROWP")
    nc.gpsimd.memset(ROWP[:, 0:QT], PEN)
    nc.gpsimd.memset(ROWP[:, QT:W], 0.0)
    RMASK = _t([128, W], "RMASK")
    nc.gpsimd.memset(RMASK[:, 0:QT], 1.0)
    nc.gpsimd.memset(RMASK[:, QT:W], 0.0)

    # ---- norm broadcast + plane constants
    NB = _t([128, 4], "NB")
    nc.gpsimd.partition_broadcast(NB[:], norm_sb[:], channels=128)
    nsq = _t([128, 4], "nsq")
    nc.vector.tensor_tensor(nsq[:], NB[:], NB[:], op=OP.mult)
    snn = _t([128, 1], "snn")
    nc.vector.tensor_reduce(snn[:], nsq[:, 0:3], axis=AX.X, op=OP.add)
    inv_nn = _t([128, 1], "inv_nn")
    nc.vector.reciprocal(inv_nn[:], snn[:])
    ninv2 = _t([128, 1], "ninv2")
    nc.scalar.mul(ninv2[:], inv_nn[:], -2.0)
    c4d = _t([128, 1], "c4d")
    nc.vector.tensor_tensor(c4d[:], NB[:, 3:4], inv_nn[:], op=OP.mult)
    nc.scalar.mul(c4d[:], c4d[:], 4.0)
    NINV2R = _t([128, W], "NINV2R")
    nc.vector.tensor_scalar(NINV2R[:], RMASK[:], ninv2[:], None, op0=OP.mult)
    C4DR = _t([128, W], "C4DR")
    nc.scalar.mul(C4DR[:], RMASK[:], c4d[:])

    # ---- merged plane eval: s = p.n + d, m1 = (s<0)
    s_all = _t([128, W], "s_all")
    t1_ = _t([128, W], "t1_")
    nc.scalar.mul(s_all[:], PA[:, :, 0], NB[:, 0:1])
    nc.scalar.mul(t1_[:], PA[:, :, 1], NB[:, 1:2])
    nc.vector.tensor_tensor(s_all[:], s_all[:], t1_[:], op=OP.add)
    nc.scalar.mul(t1_[:], PA[:, :, 2], NB[:, 2:3])
    nc.vector.tensor_tensor(s_all[:], s_all[:], t1_[:], op=OP.add)
    nc.vector.tensor_scalar_add(s_all[:], s_all[:], NB[:, 3:4])
    M1 = _t([128, W], "M1")
    nc.vector.tensor_scalar(M1[:], s_all[:], 0.0, None, op0=OP.is_lt)

    # ---- operand vectors V = alpha*n + beta.p (rows: reflect, cols: -2p)
    alpha = _t([128, W], "alpha")
    nc.vector.tensor_tensor(alpha[:], s_all[:], NINV2R[:], op=OP.mult)
    V = []
    for c in range(3):
        tv = _t([128, W], f"tv{c}")
        nc.scalar.mul(tv[:], alpha[:], NB[:, c : c + 1])
        tb2 = _t([128, W], f"tb2{c}")
        nc.gpsimd.tensor_tensor(tb2[:], BETA[:], PA[:, :, c], op=OP.mult)
        v = _t([128, W], f"v{c}")
        nc.vector.tensor_tensor(v[:], tv[:], tb2[:], op=OP.add)
        V.append(v)

    # ---- rr = |p|^2 + (4d/nn)*s*is_row + penalty
    pp = _t([128, W], "pp")
    nc.vector.tensor_tensor(pp[:], PA[:, :, 0], PA[:, :, 0], op=OP.mult)
    q1 = _t([128, W], "q1")
    nc.gpsimd.tensor_tensor(q1[:], PA[:, :, 1], PA[:, :, 1], op=OP.mult)
    nc.vector.tensor_tensor(pp[:], pp[:], q1[:], op=OP.add)
    q2 = _t([128, W], "q2")
    nc.gpsimd.tensor_tensor(q2[:], PA[:, :, 2], PA[:, :, 2], op=OP.mult)
    nc.vector.tensor_tensor(pp[:], pp[:], q2[:], op=OP.add)
    t3 = _t([128, W], "t3")
    nc.vector.tensor_tensor(t3[:], s_all[:], C4DR[:], op=OP.mult)
    t4 = _t([128, W], "t4")
    nc.gpsimd.tensor_tensor(t4[:], M1[:], SGNP[:], op=OP.mult)
    nc.gpsimd.tensor_tensor(t4[:], t4[:], ROWP[:], op=OP.add)
    rr = _t([128, W], "rr")
    nc.vector.tensor_tensor(rr[:], pp[:], t3[:], op=OP.add)
    nc.vector.tensor_tensor(rr[:], rr[:], t4[:], op=OP.add)

    # ---- bf16 hi/lo splits into one composite: slots [Vh0-2, Vl0-2, rrh, rrl]
    ACOMP = _t([128, 8, W], "ACOMP", BF)

    def split_into(src, hs, ls, name):
        nc.scalar.copy(ACOMP[:, hs, :], src[:])
        hf = _t([128, W], f"hf_{name}")
        nc.vector.tensor_scalar(hf[:], ACOMP[:, hs, :], 1.0, None, op0=OP.mult)
        lr = _t([128, W], f"lr_{name}")
        nc.gpsimd.tensor_tensor(lr[:], src[:], hf[:], op=OP.subtract)
        nc.scalar.copy(ACOMP[:, ls, :], lr[:])

    for c in range(3):
        split_into(V[c], c, 3 + c, f"v{c}")
    split_into(rr, 6, 7, "rr")

    # ---- K-major operand images via DRAM round trip (rearranged dests)
    stgA = drm.tile([16, 128, QT], BF, name="stgA")
    stgB = drm.tile([16, 128, QC], BF, name="stgB")

    AHI = ACOMP[:, 0:3, 0:QT]
    ALO = ACOMP[:, 3:6, 0:QT]
    ARR = ACOMP[:, 6:8, 0:QT]
    nc.sync.dma_start(stgA[0:3, :, :].rearrange("k i m -> i k m"), AHI)
    nc.scalar.dma_start(stgA[3:6, :, :].rearrange("k i m -> i k m"), AHI)
    nc.sync.dma_start(stgA[6:9, :, :].rearrange("k i m -> i k m"), ALO)
    nc.scalar.dma_start(stgA[9:12, :, :].rearrange("k i m -> i k m"), ALO)
    nc.sync.dma_start(stgA[12:14, :, :].rearrange("k i m -> i k m"), ARR)
    nc.gpsimd.dma_start(stgA[14:16, :, :], ONES64[:])

    BHI = ACOMP[:, 0:3, QT:W]
    BLO = ACOMP[:, 3:6, QT:W]
    BRR = ACOMP[:, 6:8, QT:W]
    nc.scalar.dma_start(stgB[0:3, :, :].rearrange("k p q -> p k q"), BHI)
    nc.gpsimd.dma_start(stgB[3:6, :, :].rearrange("k p q -> p k q"), BLO)
    nc.gpsimd.dma_start(stgB[6:9, :, :].rearrange("k p q -> p k q"), BHI)
    nc.gpsimd.dma_start(stgB[9:12, :, :].rearrange("k p q -> p k q"), BLO)
    nc.gpsimd.dma_start(stgB[12:14, :, :], ONES64[:, 0:8])
    nc.scalar.dma_start(stgB[14:16, :, :].rearrange("k p q -> p k q"), BRR)

    TA = _t([16, 128, QT], "TA", BF)
    nc.sync.dma_start(TA[:], stgA[:])
    TB = _t([16, 128, QC], "TB", BF)
    nc.scalar.dma_start(TB[:], stgB[:])

    # ---- masks: M1 tile-layout via DRAM reshuffle; m2 as (1,512) row
    stgQ = drm.tile([QT, 128], FP, name="stgQ")
    nc.gpsimd.dma_start(stgQ[:], M1[:, 0:QT])
    M1t = _t([128, QT], "M1t")
    nc.gpsimd.dma_start(M1t[:], stgQ[:].rearrange("g p -> p g"))
    M2CB = _t([128, QC], "M2CB")
    nc.vector.tensor_scalar(M2CB[:], M1[:, QT:W], -1.0, 1.0, op0=OP.mult, op1=OP.add)
    stgM = drm.tile([128, QC], FP, name="stgM")
    nc.gpsimd.dma_start(stgM[:], M2CB[:])
    M2row = _t([1, 512], "M2row")
    nc.gpsimd.dma_start(M2row[:], stgM[:])

    # ---- c1/c2 + reciprocals precomputed before the collective
    c1row = _t([128, 1], "c1row")
    nc.vector.tensor_reduce(c1row[:], M1[:, 0:QT], axis=AX.X, op=OP.add)
    c1_ps = pss.tile([1, 1], FP, tag="ps")
    nc.tensor.matmul(c1_ps[:], c1row[:], ones_c[:], start=True, stop=True)
    c1 = _t([1, 1], "c1")
    nc.scalar.copy(c1[:], c1_ps[:])
    c2 = _t([1, 1], "c2")
    nc.vector.tensor_scalar(c2[:], c1[:], -1.0, float(N), op0=OP.mult, op1=OP.add)
    nc.vector.tensor_scalar_max(c1[:], c1[:], 1.0)
    nc.vector.tensor_scalar_max(c2[:], c2[:], 1.0)
    rc1 = _t([1, 1], "rc1")
    nc.vector.reciprocal(rc1[:], c1[:])
    rc2 = _t([1, 1], "rc2")
    nc.vector.reciprocal(rc2[:], c2[:])

    # ---- main loop: FS = -(F tile) in fp16; row-max partials + col-max acc
    CM = _t([128, 512], "CM", HF)
    nc.gpsimd.memset(CM[:], CMINIT)
    D2 = _t([128, QT], "D2", HF)

    for m in range(QT):
        fps = psf.tile([128, 512], FP, name="fps")
        nc.tensor.matmul(
            fps[:], TA[:, 4 * m : 4 * (m + 1), :], TB[:], start=True, stop=True
        )
        FS = fsp.tile([128, 512], HF, name="FS")
        nc.scalar.mul(FS[:], fps[:], -1.0)
        nc.vector.tensor_reduce(D2[:, m : m + 1], FS[:], axis=AX.X, op=OP.max)
        nc.vector.tensor_tensor(CM[:], CM[:], FS[:], op=OP.max)

    # ---- columns: d1 = max over partitions, then masked sum s1
    D1B = _t([128, 512], "D1B")
    nc.gpsimd.partition_all_reduce(D1B[:], CM[:], 128, bass_isa.ReduceOp.max)
    w1 = _t([1, 512], "w1")
    nc.vector.tensor_tensor(w1[:], D1B[0:1, :], M2row[:], op=OP.mult)
    s1 = _t([1, 1], "s1")
    nc.vector.tensor_reduce(s1[:], w1[:], axis=AX.X, op=OP.add)

    # encode s1 into partition slot column: slot[core_id] = s1, else -BIG
    s1b = _t([128, 1], "s1b")
    nc.gpsimd.partition_broadcast(s1b[:], s1[:], channels=128)
    slot = _t([128, 1], "slot")
    nc.vector.tensor_tensor(slot[:], s1b[:], OH[:], op=OP.mult)
    bm = _t([128, 1], "bm")
    nc.vector.tensor_scalar(bm[:], OH[:], BIG, -BIG, op0=OP.mult, op1=OP.add)
    nc.vector.tensor_tensor(slot[:], slot[:], bm[:], op=OP.add)

    D2f = _t([128, QT], "D2f")
    nc.scalar.copy(D2f[:], D2[:])

    # ---- AllReduce(max) of [D2 | slot] over all 8 cores
    pay = drm.tile([128, QT + 1], FP, name="pay")
    pay2 = drm.tile([128, QT + 1], FP, name="pay2")
    nc.gpsimd.dma_start(pay[:, 0:QT], D2f[:])
    nc.gpsimd.dma_start(pay[:, QT : QT + 1], slot[:])
    nc.gpsimd.collective_compute(
        "AllReduce",
        OP.max,
        replica_groups=[list(range(NCORES))],
        ins=[pay.opt()],
        outs=[pay2.opt()],
    )
    G2 = _t([128, QT], "G2")
    nc.gpsimd.dma_start(G2[:], pay2[:, 0:QT])
    slots = _t([128, 1], "slots")
    nc.gpsimd.dma_start(slots[:], pay2[:, QT : QT + 1])

    # ---- finish: s2 = sum(G2*m1t), sum slots, combine with rc1/rc2
    w2 = _t([128, QT], "w2")
    nc.vector.tensor_tensor(w2[:], G2[:], M1t[:], op=OP.mult)
    w2s = _t([128, 1], "w2s")
    nc.vector.tensor_reduce(w2s[:], w2[:], axis=AX.X, op=OP.add)
    s2_ps = pss.tile([1, 1], FP, tag="ps")
    nc.tensor.matmul(s2_ps[:], w2s[:], ones_c[:], start=True, stop=True)

    sa_ps = pss.tile([1, 1], FP, tag="ps")
    nc.tensor.matmul(
        sa_ps[:], slots[0:NCORES, :], ones_c[0:NCORES, :], start=True, stop=True
    )

    s2 = _t([1, 1], "s2")
    nc.scalar.copy(s2[:], s2_ps[:])
    sum_s1 = _t([1, 1], "sum_s1")
    nc.scalar.copy(sum_s1[:], sa_ps[:])
    av2 = _t([1, 1], "av2")
    nc.vector.tensor_tensor(av2[:], s2[:], rc1[:], op=OP.mult)
    av1 = _t([1, 1], "av1")
    nc.vector.tensor_tensor(av1[:], sum_s1[:], rc2[:], op=OP.mult)
    res = _t([1, 1], "res")
    nc.vector.tensor_tensor(res[:], av1[:], av2[:], op=OP.add)
    nc.scalar.mul(res[:], res[:], -50.0)
    nc.sync.dma_start(out_ap[:], res[:])

    for p in (psf, pss, per, fsp, drm):
        p.seal()


_NC = None


def build():
    global _NC
    if _NC is not None:
        return _NC
    nc = bacc.Bacc(
        "TRN2", target_bir_lowering=False, debug=False, num_devices=NCORES
    )
    norm_ap = nc.dram_tensor("norm4", [1, 4], FP, kind="ExternalInput").ap()
    pa_ap = nc.dram_tensor("pa", [128, W, 3], FP, kind="ExternalInput").ap()
    oh_ap = nc.dram_tensor("oh", [128, 1], FP, kind="ExternalInput").ap()
    out_ap = nc.dram_tensor("out", [1, 1], FP, kind="ExternalOutput").ap()
    with tile.TileContext(nc) as tc:
        _emit(tc, out_ap, norm_ap, pa_ap, oh_ap)
    nc.compile()
    _NC = nc
    return nc


def make_in_maps(norm, points):
    norm = np.ascontiguousarray(norm, dtype=np.float32)
    pts = np.ascontiguousarray(points, dtype=np.float32)
    PTq = pts.reshape(128, QT, 3)
    maps = []
    for c in range(NCORES):
        oh = np.zeros((128, 1), np.float32)
        oh[c, 0] = 1.0
        cb = pts[512 * c : 512 * (c + 1)].reshape(128, QC, 3)
        pa = np.ascontiguousarray(np.concatenate([PTq, cb], axis=1))
        maps.append({"norm4": norm, "pa": pa, "oh": oh})
    return maps


LAST_RESULTS = None


def kernel(norm, points):
    global LAST_RESULTS
    nc = build()
    maps = make_in_maps(norm, points)
    trace = bool(os.environ.get("KERNEL_TRACE"))
    LAST_RESULTS = run_bass_kernel_spmd(
        nc, maps, list(range(NCORES)), trace=trace
    )
    out = np.asarray(LAST_RESULTS.results[0]["out"], dtype=np.float32)
    return out.reshape(())



# revision 6
# speedup vs baseline: 2.7272x; 2.7272x over previous
"""Chamfer-style point loss (nn_PointLoss) on 8 Trainium2 NeuronCores.

Math (reference): reflect points across plane n.x+d=0; half1 = reflected
points (valid where s=p.n+d < 0, mask m1), half2 = original points (mask
m2 = ~m1). D[i,j] = ||half1[i]-half2[j]||^2. Output scalar =
50*(sum_j min_i(D) m2_j / c2 + sum_i min_j(D) m1_i / c1).

v4 formulation: each core computes TWO blocks of the negated-penalized
distance matrix -(D + P*rowpen + P*colpen):
  block A: its own 512 rows x all 4096 cols  -> row-maxes are fully
           local -> masked sum s2_c (sentinel filter, no mask tensors)
  block B: all 4096 rows x its own 512 cols  -> col-maxes are fully
           local -> masked sum s1_c
so each core emits one partial scalar out_c = -50*(s1_c/c2 + s2_c/c1)
and the only cross-core step is an 8-way scalar sum (host gather by
default, optional 4-byte on-device AllReduce via USE_AR).

Device pipeline: per-core input = full point set ROLLED so its own 512
points occupy groups 0..3 (p-inner layout, point = g*128+p); prep builds
negated reflected coords (A side), -2p (B side), rr/cc rank-1 rows with
penalties, all hi/lo bf16-split into two [128,4,16,8] composites; four
xbar DMA-transposes per side produce K-major images [16, 32, 128]; 64
K=13 bf16 matmuls (interleaved A/B) stream into fp32 PSUM; DVE reduces
block-A tiles straight from PSUM, ACT converts block-B tiles to fp16 and
GpSimd max-accumulates column mins. PE is pre-warmed with zero matmuls
(results folded into the output as +0 to defeat DCE).

Sharding: both row- and column- 8-way sharding are used (doubled
compute); all cross-core reduction collapses to the scalar sum.
"""

import os
import sys

import numpy as np

for _p in ("/opt/trn_rl_repo", "/root/.axon_site/_ro/trn_rl_repo"):
    if os.path.isdir(_p) and _p not in sys.path:
        sys.path.insert(0, _p)

import concourse.bacc as bacc
import concourse.bass_isa as bass_isa
import concourse.tile as tile
from concourse import mybir
from concourse.bass_utils import run_bass_kernel_spmd

FP = mybir.dt.float32
BF = mybir.dt.bfloat16
HF = mybir.dt.float16
AX = mybir.AxisListType
OP = mybir.AluOpType

N = 4096
NCORES = 8
G = 32              # groups of 128 points; point index = g*128 + p
PEN = float(2**14)  # row/col penalty, keeps penalized -(F) finite in fp16
SENT = -8000.0      # sentinel threshold: valid maxes are > -1000
CMINIT = -60000.0
NWARM = 10
USE_AR = False      # False: host sums 8 partial scalars; True: device AllReduce


def _emit(tc, out_ap, norm_ap, pa_ap):
    nc = tc.nc

    psf = tc.alloc_tile_pool(name="psf", bufs=3, space="PSUM")
    pss = tc.alloc_tile_pool(name="pss", bufs=1, space="PSUM")
    pwd = tc.alloc_tile_pool(name="pwd", bufs=1, space="PSUM")
    per = tc.alloc_tile_pool(name="per", bufs=1)
    fsp = tc.alloc_tile_pool(name="fsp", bufs=3)
    drm = tc.alloc_tile_pool(name="drm", bufs=1, space="DRAM")

    def _t(shape, name, dt=FP):
        return per.tile(shape, dt, name=name)

    # ---- memsets spread over gpsimd/vector while inputs stream in
    ZW = _t([16, 640], "ZW", BF)
    nc.gpsimd.memset(ZW[:], 0.0)
    CALL_A = _t([128, 16, 32], "CALL_A", BF)
    nc.gpsimd.memset(CALL_A[:], 0.0)
    nc.gpsimd.memset(CALL_A[:, 11:13, :], -1.0)
    ones_c = _t([128, 1], "ones_c")
    nc.gpsimd.memset(ones_c[:], 1.0)
    CALL_B = _t([128, 16, 32], "CALL_B", BF)
    nc.vector.memset(CALL_B[:], 0.0)
    nc.vector.memset(CALL_B[:, 9:11, :], 1.0)
    CM = _t([128, 512], "CM", HF)
    nc.vector.memset(CM[:], CMINIT)

    # ---- PE warmup: spin the tensor clock up during prep; zeros so the
    # result can be added to the output (defeats DCE) without effect.
    pw_ = pwd.tile([128, 512], FP, name="pw")
    for _ in range(NWARM):
        nc.tensor.matmul(pw_[:], ZW[:, 0:128], ZW[:, 128:640], start=True, stop=True)
    pwsb = _t([1, 1], "pwsb")
    nc.scalar.copy(pwsb[:], pw_[0:1, 0:1])

    # ---- inputs
    norm_sb = _t([1, 4], "norm_sb")
    nc.sync.dma_start(norm_sb[:], norm_ap[:])
    PA = _t([128, 3, 32], "PA")
    nc.sync.dma_start(PA[:, 0:2, :], pa_ap[:, 0:2, :])
    nc.scalar.dma_start(PA[:, 2:3, :], pa_ap[:, 2:3, :])

    # ---- plane constants
    NB = _t([128, 4], "NB")
    nc.gpsimd.partition_broadcast(NB[:], norm_sb[:], channels=128)
    nsq = _t([128, 4], "nsq")
    nc.vector.tensor_tensor(nsq[:], NB[:], NB[:], op=OP.mult)
    snn = _t([128, 1], "snn")
    nc.vector.tensor_reduce(snn[:], nsq[:, 0:3], axis=AX.X, op=OP.add)
    inv_nn = _t([128, 1], "inv_nn")
    nc.vector.reciprocal(inv_nn[:], snn[:])
    pinv2 = _t([128, 1], "pinv2")
    nc.scalar.mul(pinv2[:], inv_nn[:], 2.0)
    c4d = _t([128, 1], "c4d")
    nc.vector.tensor_tensor(c4d[:], NB[:, 3:4], inv_nn[:], op=OP.mult)
    nc.scalar.mul(c4d[:], c4d[:], 4.0)

    # ---- s = p.n + d over all 4096 points (p-inner layout)
    s_all = _t([128, 32], "s_all")
    t1 = _t([128, 32], "t1")
    nc.scalar.mul(s_all[:], PA[:, 0, :], NB[:, 0:1])
    nc.scalar.mul(t1[:], PA[:, 1, :], NB[:, 1:2])
    nc.vector.tensor_tensor(s_all[:], s_all[:], t1[:], op=OP.add)
    nc.scalar.mul(t1[:], PA[:, 2, :], NB[:, 2:3])
    nc.vector.tensor_tensor(s_all[:], s_all[:], t1[:], op=OP.add)
    nc.vector.tensor_scalar_add(s_all[:], s_all[:], NB[:, 3:4])
    M1f = _t([128, 32], "M1f")
    nc.vector.tensor_scalar(M1f[:], s_all[:], 0.0, None, op0=OP.is_lt)
    pw1 = _t([128, 32], "pw1")
    nc.gpsimd.tensor_scalar_mul(pw1[:], M1f[:], PEN)

    # ---- c1/c2 + reciprocals (identical on every core)
    c1row = _t([128, 1], "c1row")
    nc.vector.tensor_reduce(c1row[:], M1f[:], axis=AX.X, op=OP.add)
    c1ps = pss.tile([1, 1], FP, tag="ps")
    nc.tensor.matmul(c1ps[:], c1row[:], ones_c[:], start=True, stop=True)
    c1 = _t([1, 1], "c1")
    nc.scalar.copy(c1[:], c1ps[:])
    c2 = _t([1, 1], "c2")
    nc.vector.tensor_scalar(c2[:], c1[:], -1.0, float(N), op0=OP.mult, op1=OP.add)
    nc.vector.tensor_scalar_max(c1[:], c1[:], 1.0)
    nc.vector.tensor_scalar_max(c2[:], c2[:], 1.0)
    rc1 = _t([1, 1], "rc1")
    nc.vector.reciprocal(rc1[:], c1[:])
    rc2 = _t([1, 1], "rc2")
    nc.vector.reciprocal(rc2[:], c2[:])

    # ---- |p|^2
    pp = _t([128, 32], "pp")
    q1 = _t([128, 32], "q1")
    q2 = _t([128, 32], "q2")
    nc.vector.tensor_tensor(pp[:], PA[:, 0, :], PA[:, 0, :], op=OP.mult)
    nc.gpsimd.tensor_tensor(q1[:], PA[:, 1, :], PA[:, 1, :], op=OP.mult)
    nc.gpsimd.tensor_tensor(q2[:], PA[:, 2, :], PA[:, 2, :], op=OP.mult)
    nc.vector.tensor_tensor(pp[:], pp[:], q1[:], op=OP.add)
    nc.vector.tensor_tensor(pp[:], pp[:], q2[:], op=OP.add)

    # ---- A side: an = -(reflected p) = (2s/nn)*n - p   (negated so PSUM
    # holds -(D+pen) and mins become maxes)
    nalpha = _t([128, 32], "nalpha")
    nc.scalar.mul(nalpha[:], s_all[:], pinv2[:])
    AN = _t([128, 3, 32], "AN")
    for c in range(3):
        tv = _t([128, 32], f"tv{c}")
        nc.scalar.mul(tv[:], nalpha[:], NB[:, c : c + 1])
        nc.vector.tensor_tensor(AN[:, c, :], tv[:], PA[:, c, :], op=OP.subtract)
    # B side: bn = -2p
    BN = _t([128, 3, 32], "BN")
    nc.vector.tensor_scalar_mul(BN[:], PA[:], -2.0)

    # ---- rank-1 rows: rrA_neg = -(|a|^2 + P*(1-m1)), ccB = |p|^2 + P*m1
    t3 = _t([128, 32], "t3")
    nc.scalar.mul(t3[:], s_all[:], c4d[:])
    u_ = _t([128, 32], "u_")
    nc.vector.tensor_tensor(u_[:], pp[:], t3[:], op=OP.add)  # |a|^2
    v_ = _t([128, 32], "v_")
    nc.gpsimd.tensor_scalar(v_[:], pw1[:], 1.0, -PEN, op0=OP.mult, op1=OP.add)
    RRn = _t([128, 32], "RRn")
    nc.vector.tensor_tensor(RRn[:], v_[:], u_[:], op=OP.subtract)
    CCp = _t([128, 32], "CCp")
    nc.gpsimd.tensor_tensor(CCp[:], pp[:], pw1[:], op=OP.add)

    # ---- hi/lo bf16 splits into the two K-composites
    # K pairing: [aH*bH (0:3), aH*bL (3:6), aL*bH (6:9), rrh*1 (9),
    #             rrl*1 (10), -1*cch (11), -1*ccl (12), zeros (13:16)]
    HFA = _t([128, 3, 32], "HFA")
    LOA = _t([128, 3, 32], "LOA")
    nc.scalar.copy(CALL_A[:, 0:3, :], AN[:])
    nc.gpsimd.tensor_scalar_mul(CALL_A[:, 3:6, :], AN[:], 1.0)
    nc.vector.tensor_scalar_mul(HFA[:], CALL_A[:, 0:3, :], 1.0)
    nc.gpsimd.tensor_tensor(LOA[:], AN[:], HFA[:], op=OP.subtract)
    nc.scalar.copy(CALL_A[:, 6:9, :], LOA[:])

    HFR = _t([128, 32], "HFR")
    LOR = _t([128, 32], "LOR")
    nc.scalar.copy(CALL_A[:, 9, :], RRn[:])
    nc.vector.tensor_scalar_mul(HFR[:], CALL_A[:, 9, :], 1.0)
    nc.gpsimd.tensor_tensor(LOR[:], RRn[:], HFR[:], op=OP.subtract)
    nc.scalar.copy(CALL_A[:, 10, :], LOR[:])

    HFB = _t([128, 3, 32], "HFB")
    LOB = _t([128, 3, 32], "LOB")
    nc.scalar.copy(CALL_B[:, 0:3, :], BN[:])
    nc.gpsimd.tensor_scalar_mul(CALL_B[:, 6:9, :], BN[:], 1.0)
    nc.vector.tensor_scalar_mul(HFB[:], CALL_B[:, 0:3, :], 1.0)
    nc.gpsimd.tensor_tensor(LOB[:], BN[:], HFB[:], op=OP.subtract)
    nc.scalar.copy(CALL_B[:, 3:6, :], LOB[:])

    HFC = _t([128, 32], "HFC")
    LOC = _t([128, 32], "LOC")
    nc.scalar.copy(CALL_B[:, 11, :], CCp[:])
    nc.vector.tensor_scalar_mul(HFC[:], CALL_B[:, 11, :], 1.0)
    nc.gpsimd.tensor_tensor(LOC[:], CCp[:], HFC[:], op=OP.subtract)
    nc.scalar.copy(CALL_B[:, 12, :], LOC[:])

    # ---- K-major images via DRAM round trip, chunked by i-range so the
    # loop starts as soon as the first chunks land (64B runs on write,
    # contiguous read-back)
    stgA = drm.tile([16, 128, 32], BF, name="stgA")
    stgB = drm.tile([16, 128, 32], BF, name="stgB")
    TAF = _t([16, 128, 32], "TAF", BF)
    TBF = _t([16, 128, 32], "TBF", BF)
    xq = [nc.sync, nc.scalar]
    for r in range(4):
        sl = slice(32 * r, 32 * (r + 1))
        xq[r % 2].dma_start(
            stgA[:, sl, :].rearrange("k i s -> i k s"), CALL_A[sl, :, :]
        )
        xq[(r + 1) % 2].dma_start(
            stgB[:, sl, :].rearrange("k i s -> i k s"), CALL_B[sl, :, :]
        )
    for r in range(4):
        sl = slice(32 * r, 32 * (r + 1))
        xq[r % 2].dma_start(TAF[:, sl, :], stgA[:, sl, :])
        xq[(r + 1) % 2].dma_start(TBF[:, sl, :], stgB[:, sl, :])

    # ---- main loop: 64 matmuls, interleaved block B (col dir) and block
    # A (row dir). PSUM holds -(D + penalties). Tile m covers rows/cols
    # made of points {128m..128m+127} (i-chunks of the shared images).
    D2A = _t([128, 4, 8], "D2A")
    for i in range(G):
        pb = psf.tile([128, 512], FP, name="pb")
        nc.tensor.matmul(
            pb[:], TAF[:, 4 * i : 4 * (i + 1), :], TBF[:, 0:16, :],
            start=True, stop=True,
        )
        FS = fsp.tile([128, 512], HF, name="FS")
        nc.scalar.copy(FS[:], pb[:])
        nc.vector.tensor_tensor(CM[:], CM[:], FS[:], op=OP.max)

        g, j = i % 4, i // 4
        pa_ = psf.tile([128, 512], FP, name="pa")
        nc.tensor.matmul(
            pa_[:], TAF[:, 4 * g : 4 * (g + 1), :],
            TBF[:, 16 * j : 16 * (j + 1), :],
            start=True, stop=True,
        )
        nc.vector.tensor_reduce(D2A[:, g, j : j + 1], pa_[:], axis=AX.X, op=OP.max)

    # ---- block A finish: d2 per own row, sentinel mask, sum
    d2g = _t([128, 4], "d2g")
    nc.vector.tensor_reduce(d2g[:], D2A[:], axis=AX.X, op=OP.max)
    msk2 = _t([128, 4], "msk2")
    nc.vector.tensor_scalar(msk2[:], d2g[:], SENT, None, op0=OP.is_gt)
    w2 = _t([128, 4], "w2")
    nc.vector.tensor_tensor(w2[:], msk2[:], d2g[:], op=OP.mult)
    w2s = _t([128, 1], "w2s")
    nc.vector.tensor_reduce(w2s[:], w2[:], axis=AX.X, op=OP.add)
    s2ps = pss.tile([1, 1], FP, tag="ps")
    nc.tensor.matmul(s2ps[:], w2s[:], ones_c[:], start=True, stop=True)
    s2 = _t([1, 1], "s2")
    nc.scalar.copy(s2[:], s2ps[:])

    # ---- block B finish: col-min across partitions, sentinel mask, sum
    D1B = _t([128, 512], "D1B")
    nc.gpsimd.partition_all_reduce(D1B[:], CM[:], 128, bass_isa.ReduceOp.max)
    msk1 = _t([1, 512], "msk1")
    nc.vector.tensor_scalar(msk1[:], D1B[0:1, :], SENT, None, op0=OP.is_gt)
    w1 = _t([1, 512], "w1")
    nc.vector.tensor_tensor(w1[:], msk1[:], D1B[0:1, :], op=OP.mult)
    s1 = _t([1, 1], "s1")
    nc.vector.tensor_reduce(s1[:], w1[:], axis=AX.X, op=OP.add)

    # ---- combine: out_c = -50*(s1/c2 + s2/c1) (+ 0 from warmup)
    a1 = _t([1, 1], "a1")
    nc.vector.tensor_tensor(a1[:], s1[:], rc2[:], op=OP.mult)
    a2 = _t([1, 1], "a2")
    nc.vector.tensor_tensor(a2[:], s2[:], rc1[:], op=OP.mult)
    res = _t([1, 1], "res")
    nc.vector.tensor_tensor(res[:], a1[:], a2[:], op=OP.add)
    nc.scalar.mul(res[:], res[:], -50.0)
    nc.vector.tensor_tensor(res[:], res[:], pwsb[:], op=OP.add)

    if USE_AR:
        pay = drm.tile([1, 1], FP, name="pay")
        pay2 = drm.tile([1, 1], FP, name="pay2")
        nc.gpsimd.dma_start(pay[:], res[:])
        nc.gpsimd.collective_compute(
            "AllReduce",
            OP.add,
            replica_groups=[list(range(NCORES))],
            ins=[pay.opt()],
            outs=[pay2.opt()],
        )
        res2 = _t([1, 1], "res2")
        nc.gpsimd.dma_start(res2[:], pay2[:])
        nc.sync.dma_start(out_ap[:], res2[:])
    else:
        nc.sync.dma_start(out_ap[:], res[:])

    for p in (psf, pss, pwd, per, fsp, drm):
        p.seal()


_NC = None


def build():
    global _NC
    if _NC is not None:
        return _NC
    nc = bacc.Bacc(
        "TRN2", target_bir_lowering=False, debug=False, num_devices=NCORES
    )
    norm_ap = nc.dram_tensor("norm4", [1, 4], FP, kind="ExternalInput").ap()
    pa_ap = nc.dram_tensor("pa", [128, 3, 32], FP, kind="ExternalInput").ap()
    out_ap = nc.dram_tensor("out", [1, 1], FP, kind="ExternalOutput").ap()
    with tile.TileContext(nc) as tc:
        _emit(tc, out_ap, norm_ap, pa_ap)
    nc.compile()
    _NC = nc
    return nc


def make_in_maps(norm, points):
    norm = np.ascontiguousarray(norm, dtype=np.float32).reshape(1, 4)
    pts = np.ascontiguousarray(points, dtype=np.float32)
    maps = []
    for c in range(NCORES):
        rolled = np.concatenate([pts[512 * c :], pts[: 512 * c]], axis=0)
        pa = rolled.reshape(128, G, 3).transpose(0, 2, 1)  # [128, 3, 32]
        maps.append({"norm4": norm, "pa": np.ascontiguousarray(pa)})
    return maps


def combine_outs(outs):
    if USE_AR:
        return np.float32(outs[0])
    return np.float32(np.sum(np.asarray(outs, dtype=np.float64)))


LAST_RESULTS = None


def kernel(norm, points):
    global LAST_RESULTS
    nc = build()
    maps = make_in_maps(norm, points)
    trace = bool(os.environ.get("KERNEL_TRACE"))
    LAST_RESULTS = run_bass_kernel_spmd(
        nc, maps, list(range(NCORES)), trace=trace
    )
    outs = [
        np.asarray(r["out"], dtype=np.float32).reshape(())
        for r in LAST_RESULTS.results
    ]
    return np.asarray(combine_outs(outs), dtype=np.float32).reshape(())
